# revision 1
# baseline (speedup 1.0000x reference)
"""Distributed Trainium2 kernel for cross-attention (nn_Attention_50732153701013).

Reference computation (b=2, n=2048, dim=1024, heads=16, d_head=64):
    qkv  = split(x  @ W_qkv)          -> q,  k,  v
    qkv1 = split(x1 @ W_qkv)          -> q1, k1, v1
    out  = merge(softmax(q  k1^T / 8) v1) @ W_out + b_out
    out1 = merge(softmax(q1 k ^T / 8) v ) @ W_out + b_out

Sharding over 8 cores: core c handles batch (c // 4) and heads
[(c%4)*4, (c%4)*4+4).  Each core computes its 4 heads' attention for both
cross directions plus the partial out-projection (row-slice of W_out);
the host sums the 4 partial outputs per batch.

Device-side layout notes:
  * x/x1 are pre-transposed on the host to xT [dim, n] so every matmul
    contraction (over dim / d_head / n) has its axis on SBUF partitions.
  * Scores are computed transposed: S^T[m, n] = k[m]·q[n], so the softmax
    reduction axis (m) lies on PSUM partitions.  exp() needs no max
    subtraction (|scores| <~ 6).  The softmax denominator is obtained by
    appending a ones-column to V, so the AV matmul also yields
    colsum(exp S^T) as PSUM row 64.  The division happens on the 64-row
    O^T tile: reciprocal of the colsum row, broadcast across partitions
    with a PE outer product (ones[1,64]^T x recip[1,512]).
  * Matmul operands are bf16 (1 PE cycle/row + fast weight load; fp32 runs
    the HI/LO path at less than half throughput).  PSUM accumulation stays
    fp32.  Measured end-to-end relative error ~6e-3, gate is 2e-2.
  * The out-projection of block nb is emitted one attention unit late so
    its TensorE burst hides inside the Scalar engine's exp slack instead
    of stalling the softmax pipeline at block boundaries.
"""

import numpy as np

B, N, DIM = 2, 2048, 1024
HEADS, DHEAD = 16, 64
H_LOC = 4                 # heads per core
INNER_LOC = H_LOC * DHEAD  # 256
NCORES = 8
SCALE = DHEAD ** -0.5     # 0.125

_CACHED = {}


def _build_graph():
    import concourse.mybir as mybir
    from concourse import bacc
    from concourse.tile import TileContext

    f32 = mybir.dt.float32
    bf16 = mybir.dt.bfloat16
    AF = mybir.ActivationFunctionType

    nc = bacc.Bacc(None, target_bir_lowering=False)

    xT = nc.dram_tensor("xT", [DIM, N], f32, kind="ExternalInput")
    x1T = nc.dram_tensor("x1T", [DIM, N], f32, kind="ExternalInput")
    wqkv = nc.dram_tensor("wqkv", [DIM, 3 * INNER_LOC], f32, kind="ExternalInput")
    wout = nc.dram_tensor("wout", [INNER_LOC, DIM], f32, kind="ExternalInput")
    out = nc.dram_tensor("out", [2, N, DIM], f32, kind="ExternalOutput")

    KO = DIM // 128            # 8 contraction chunks for the projections
    NB = 4                     # n blocks of 512
    NT = N // 128              # 16 n tiles / m chunks
    VW = DHEAD + 1             # 65: head slice width in v_sb (data + ones col)

    with TileContext(nc) as tc:
        with (
            nc.allow_low_precision(reason="bf16 matmul operands, fp32 accum"),
            tc.tile_pool(name="persist", bufs=1) as persist,
            tc.tile_pool(name="qk", bufs=1) as qkpool,
        ):
            wqkv_sb = persist.tile([128, KO, 3 * INNER_LOC], bf16)
            nc.gpsimd.dma_start(
                wqkv_sb[:], wqkv.rearrange("(ko p) c -> p ko c", p=128)
            )
            wout_sb = persist.tile([128, 2, DIM], bf16)
            ones_f32 = persist.tile([128, 1], f32)
            nc.any.memset(ones_f32[:], 1.0)
            ones_row = persist.tile([1, 64], bf16)
            nc.vector.tensor_copy(
                ones_row[:], ones_f32[0:1, :].broadcast_to([1, 64])
            )

            # transposed q/k for both inputs: [128, chunk(2), n]
            qT = qkpool.tile([128, 2, N], bf16, tag="qT")
            kT = qkpool.tile([128, 2, N], bf16, tag="kT")
            q1T = qkpool.tile([128, 2, N], bf16, tag="q1T")
            k1T = qkpool.tile([128, 2, N], bf16, tag="k1T")
            # v in [m, head-slices] layout, ones col per head at offset 64
            v_sb = persist.tile([128, NT, H_LOC * VW], bf16, tag="v")
            v1_sb = persist.tile([128, NT, H_LOC * VW], bf16, tag="v1")
            for vt in (v_sb, v1_sb):
                nc.vector.tensor_copy(
                    vt[:].rearrange("p t (h c) -> p t h c", h=H_LOC)[:, :, :, DHEAD:],
                    ones_f32[:, None, None, :].broadcast_to([128, NT, H_LOC, 1]),
                )

            # ---------------- Stage 1: QKV projections ----------------
            with (
                tc.tile_pool(name="xstage", bufs=2) as xstage,
                tc.tile_pool(name="ps_qk", bufs=4, space="PSUM") as ps_qk,
                tc.tile_pool(name="ps_v", bufs=4, space="PSUM") as ps_v,
            ):
                for src_i, (srcT, qdst, kdst, vdst) in enumerate(
                    ((xT, qT, kT, v_sb), (x1T, q1T, k1T, v1_sb))
                ):
                    for half in range(2):
                        nslc = slice(half * 1024, (half + 1) * 1024)
                        xs = xstage.tile([128, KO, 1024], bf16, tag="xs")
                        nc.gpsimd.dma_start(
                            xs[:],
                            srcT.rearrange("(ko p) n -> p ko n", p=128)[:, :, nslc],
                        )
                        # q and k chunks ([128, 512] psum, accumulate over ko)
                        for mb in range(4):  # 0,1 -> q chunks; 2,3 -> k chunks
                            dst = qdst if mb < 2 else kdst
                            ci = mb % 2
                            for nb in range(2):
                                ps = ps_qk.tile([128, 512], f32, tag="ps_qk")
                                for ko in range(KO):
                                    nc.tensor.matmul(
                                        ps[:],
                                        wqkv_sb[:, ko, mb * 128:(mb + 1) * 128],
                                        xs[:, ko, nb * 512:(nb + 1) * 512],
                                        start=(ko == 0),
                                        stop=(ko == KO - 1),
                                    )
                                nc.vector.tensor_copy(
                                    dst[:, ci,
                                        half * 1024 + nb * 512:
                                        half * 1024 + (nb + 1) * 512],
                                    ps[:],
                                )
                        # v tiles ([n_tile 128, 256] psum)
                        for nt in range(8):
                            nt_g = half * 8 + nt
                            ps = ps_v.tile([128, INNER_LOC], f32, tag="ps_v")
                            for ko in range(KO):
                                nc.tensor.matmul(
                                    ps[:],
                                    xs[:, ko, nt * 128:(nt + 1) * 128],
                                    wqkv_sb[:, ko, 2 * INNER_LOC:3 * INNER_LOC],
                                    start=(ko == 0),
                                    stop=(ko == KO - 1),
                                )
                            nc.vector.tensor_copy(
                                vdst[:, nt_g, :]
                                .rearrange("p (h c) -> p h c", h=H_LOC)[:, :, :DHEAD],
                                ps[:].rearrange("p (h c) -> p h c", h=H_LOC),
                            )

            nc.gpsimd.dma_start(
                wout_sb[:], wout.rearrange("(ki p) d -> p ki d", p=128)
            )

            # ---------------- Stage 2: attention + out-projection ----------------
            with (
                tc.tile_pool(name="attn", bufs=4) as attn,
                tc.tile_pool(name="otp", bufs=5) as otp,
                tc.tile_pool(name="outstage", bufs=3) as outstage,
                tc.tile_pool(name="ps_s", bufs=2, space="PSUM") as ps_s,
                tc.tile_pool(name="ps_o", bufs=2, space="PSUM") as ps_o,
                tc.tile_pool(name="ps_out", bufs=1, space="PSUM") as ps_out,
                tc.tile_pool(name="ps_pb", bufs=1, space="PSUM") as ps_pb,
            ):
                ots = {}

                def attention(nb, d):
                    nslc = slice(nb * 512, (nb + 1) * 512)
                    qsrc, ksrc, vsrc = (
                        (qT, k1T, v1_sb) if d == 0 else (q1T, kT, v_sb)
                    )
                    ot = otp.tile([128, 2, 512], bf16, tag="ot")
                    ots[(nb, d)] = ot
                    for h in range(H_LOC):
                        prow = slice((h % 2) * 64, (h % 2) * 64 + 64)
                        chunk = h // 2
                        po = ps_o.tile([128, 512], f32, tag="po")
                        for mcp in range(8):
                            ps = ps_s.tile([128, 1024], f32, tag="ps_s")
                            for j in range(2):
                                mc = mcp * 2 + j
                                nc.tensor.matmul(
                                    ps[:, j * 512:(j + 1) * 512],
                                    ksrc[prow, chunk, mc * 128:(mc + 1) * 128],
                                    qsrc[prow, chunk, nslc],
                                    start=True,
                                    stop=True,
                                )
                            a = attn.tile([128, 1024], bf16, tag="a")
                            nc.scalar.activation(a[:], ps[:], AF.Exp, scale=SCALE)
                            for j in range(2):
                                mc = mcp * 2 + j
                                nc.tensor.matmul(
                                    po[0:VW, :],
                                    vsrc[:, mc, h * VW:(h + 1) * VW],
                                    a[:, j * 512:(j + 1) * 512],
                                    start=(mc == 0),
                                    stop=(mc == NT - 1),
                                )
                        csrow = attn.tile([1, 512], f32, tag="csrow")
                        nc.vector.tensor_copy(csrow[:], po[64:65, :])
                        recip_f = attn.tile([1, 512], f32, tag="recip_f")
                        nc.vector.reciprocal_approx_fast(
                            out=recip_f[:], in_=csrow[:]
                        )
                        recip = attn.tile([1, 512], bf16, tag="recip")
                        nc.vector.tensor_copy(recip[:], recip_f[:])
                        pb = ps_pb.tile([64, 512], f32, tag="ps_pb")
                        nc.tensor.matmul(pb[:], ones_row[:], recip[:],
                                         start=True, stop=True)
                        nc.vector.tensor_copy(ot[prow, chunk, :], po[0:64, :])
                        nc.vector.tensor_mul(
                            ot[prow, chunk, :], ot[prow, chunk, :], pb[:]
                        )

                def proj(nb, dirs=(0, 1), ki_split=False):
                    for d in dirs:
                        for nt in range(4):
                            ob = outstage.tile([128, DIM], f32, tag="ob")
                            for db in range(2):
                                if ki_split:
                                    for ki in range(2):
                                        ps = ps_out.tile([128, 512], f32,
                                                         tag="ps_out")
                                        nc.tensor.matmul(
                                            ps[:],
                                            ots[(nb, d)][:, ki,
                                                         nt * 128:(nt + 1) * 128],
                                            wout_sb[:, ki,
                                                    db * 512:(db + 1) * 512],
                                            start=True,
                                            stop=True,
                                        )
                                        if ki == 0:
                                            nc.vector.tensor_copy(
                                                ob[:, db * 512:(db + 1) * 512],
                                                ps[:],
                                            )
                                        else:
                                            nc.vector.tensor_add(
                                                ob[:, db * 512:(db + 1) * 512],
                                                ob[:, db * 512:(db + 1) * 512],
                                                ps[:],
                                            )
                                else:
                                    ps = ps_out.tile([128, 512], f32,
                                                     tag="ps_out")
                                    for ki in range(2):
                                        nc.tensor.matmul(
                                            ps[:],
                                            ots[(nb, d)][:, ki,
                                                         nt * 128:(nt + 1) * 128],
                                            wout_sb[:, ki,
                                                    db * 512:(db + 1) * 512],
                                            start=(ki == 0),
                                            stop=(ki == 1),
                                        )
                                    nc.vector.tensor_copy(
                                        ob[:, db * 512:(db + 1) * 512], ps[:]
                                    )
                            nc.sync.dma_start(
                                out[d, nb * 512 + nt * 128:
                                    nb * 512 + (nt + 1) * 128, :],
                                ob[:],
                            )
                    for d in dirs:
                        del ots[(nb, d)]

                # proj(nb) is emitted one attention unit late so its PE burst
                # overlaps exp work instead of stalling it.
                attention(0, 0)
                attention(0, 1)
                attention(1, 0)
                proj(0)
                attention(1, 1)
                attention(2, 0)
                proj(1)
                attention(2, 1)
                attention(3, 0)
                proj(2)
                proj(3, dirs=(0,))
                attention(3, 1)
                proj(3, dirs=(1,), ki_split=True)
    return nc


def _get_graph():
    if "nc" not in _CACHED:
        nc = _build_graph()
        # Bacc defers register allocation to finalize(); the pjrt exec path
        # serializes nc.m directly, so finalize here.
        nc.finalize()
        _CACHED["nc"] = nc
    return _CACHED["nc"]


def _make_in_maps(x, x1, W_qkv, W_out):
    in_maps = []
    for c in range(NCORES):
        b = c // 4
        h0 = (c % 4) * H_LOC
        cols = np.concatenate(
            [W_qkv[:, j * DIM + h0 * DHEAD: j * DIM + (h0 + H_LOC) * DHEAD]
             for j in range(3)],
            axis=1,
        )
        in_maps.append({
            "xT": np.ascontiguousarray(x[b].T).astype(np.float32, copy=False),
            "x1T": np.ascontiguousarray(x1[b].T).astype(np.float32, copy=False),
            "wqkv": np.ascontiguousarray(cols).astype(np.float32, copy=False),
            "wout": np.ascontiguousarray(
                W_out[h0 * DHEAD:(h0 + H_LOC) * DHEAD, :]
            ).astype(np.float32, copy=False),
        })
    return in_maps


def _run(x, x1, W_qkv, W_out, b_out, **spmd_kwargs):
    from concourse.bass_utils import run_bass_kernel_spmd

    nc = _get_graph()
    in_maps = _make_in_maps(x, x1, W_qkv, W_out)
    res = run_bass_kernel_spmd(nc, in_maps, core_ids=list(range(NCORES)),
                               **spmd_kwargs)
    parts = [r["out"].reshape(2, N, DIM) for r in res.results]
    out = np.zeros((B, N, DIM), np.float32)
    out1 = np.zeros((B, N, DIM), np.float32)
    for b in range(B):
        grp = parts[4 * b:4 * b + 4]
        out[b] = sum(p[0] for p in grp) + b_out
        out1[b] = sum(p[1] for p in grp) + b_out
    return (out, out1), res


def kernel(x, x1, W_qkv, W_out, b_out):
    x = np.asarray(x, np.float32)
    x1 = np.asarray(x1, np.float32)
    W_qkv = np.asarray(W_qkv, np.float32)
    W_out = np.asarray(W_out, np.float32)
    b_out = np.asarray(b_out, np.float32)
    (out, out1), _ = _run(x, x1, W_qkv, W_out, b_out)
    return out, out1



# revision 2
# speedup vs baseline: 1.0128x; 1.0128x over previous
"""Distributed Trainium2 kernel for cross-attention (nn_Attention_50732153701013).

Reference computation (b=2, n=2048, dim=1024, heads=16, d_head=64):
    qkv  = split(x  @ W_qkv)          -> q,  k,  v
    qkv1 = split(x1 @ W_qkv)          -> q1, k1, v1
    out  = merge(softmax(q  k1^T / 8) v1) @ W_out + b_out
    out1 = merge(softmax(q1 k ^T / 8) v ) @ W_out + b_out

Sharding over 8 cores: core c handles batch (c // 4) and heads
[(c%4)*4, (c%4)*4+4).  Host sums the 4 partial out-projections per batch.

Engine plan (per core, 33.5M softmax scores):
  * ACT exp is the hard floor: 256 x [128,1024] tiles ~ 1.11 us each
    (HW-measured) ~ 285 us.  The schedule keeps ACT continuously fed.
  * Scores as 64x64 array quads: two heads per 512-token block share the
    kT/qT pair layout (head even on partitions 0:64, odd on 64:128); four
    [64,64]-stationary matmuls (2 moving streams, 2 psum col groups) run
    concurrently: HW-measured ~238 ns per quad vs ~432 ns serial.
  * AV streams each head's exp-tile half against a V stationary with an
    appended ones column ([128,65]), so psum row 64 accumulates the
    softmax denominator for free; O^T accumulates over 16 m-chunks.
  * Normalize: DVE reciprocal of the denominator row, PE outer-product
    broadcast, DVE multiply -> ot tile (proj-ready [inner, n] layout).
  * QKV and the out-projection are emitted as 'filler' quanta between
    exp tiles so PE stays busy while ACT grinds; a need() mechanism
    forces prerequisite fillers before each attention unit.
  * Unit tails are emitted 3 steps into the NEXT unit so the serial
    denominator chain never starves ACT.
"""

import collections

import numpy as np
from ml_dtypes import bfloat16

B, N, DIM = 2, 2048, 1024
HEADS, DHEAD = 16, 64
H_LOC = 4                  # heads per core
INNER_LOC = H_LOC * DHEAD  # 256
NCORES = 8
SCALE = DHEAD ** -0.5      # 0.125

QUAD_SCORES = True

_CACHED = {}


def _build_graph():
    import concourse.mybir as mybir
    from concourse import bacc
    from concourse.tile import TileContext

    f32 = mybir.dt.float32
    bf16 = mybir.dt.bfloat16
    AF = mybir.ActivationFunctionType

    nc = bacc.Bacc(None, target_bir_lowering=False)

    xT = nc.dram_tensor("xT", [DIM, N], bf16, kind="ExternalInput")
    x1T = nc.dram_tensor("x1T", [DIM, N], bf16, kind="ExternalInput")
    wqkv = nc.dram_tensor("wqkv", [DIM, 3 * INNER_LOC], bf16,
                          kind="ExternalInput")
    wout = nc.dram_tensor("wout", [INNER_LOC, DIM], bf16,
                          kind="ExternalInput")
    out = nc.dram_tensor("out", [2, N, DIM], f32, kind="ExternalOutput")

    KO = DIM // 128            # 8 contraction chunks
    NT = N // 128              # 16 m chunks

    with TileContext(nc) as tc:
        with (
            nc.allow_low_precision(reason="bf16 matmul operands, fp32 accum"),
            tc.tile_pool(name="persist", bufs=1) as persist,
            tc.tile_pool(name="apool", bufs=12) as apool,
            tc.tile_pool(name="otp", bufs=3) as otp,
            tc.tile_pool(name="obp", bufs=3) as obp,
            tc.tile_pool(name="miscp", bufs=2) as miscp,
            tc.tile_pool(name="ps_s", bufs=2, space="PSUM") as ps_s,
            tc.tile_pool(name="ps_av", bufs=1, space="PSUM") as ps_av,
            tc.tile_pool(name="ps_mix", bufs=2, space="PSUM") as ps_mix,
        ):
            wqkv_sb = persist.tile([128, KO, 3 * INNER_LOC], bf16)
            wqkv_r = wqkv.rearrange("(ko p) c -> p ko c", p=128)
            nc.scalar.dma_start(wqkv_sb[:, :, 0:512], wqkv_r[:, :, 0:512])
            wout_sb = persist.tile([128, 2, DIM], bf16)
            ones_f32 = persist.tile([128, 1], f32)
            nc.any.memset(ones_f32[:], 1.0)
            ones_row = persist.tile([1, 64], bf16)
            nc.vector.tensor_copy(
                ones_row[:], ones_f32[0:1, :].broadcast_to([1, 64])
            )

            qT = persist.tile([128, 2, N], bf16, tag="qT")
            kT = persist.tile([128, 2, N], bf16, tag="kT")
            q1T = persist.tile([128, 2, N], bf16, tag="q1T")
            k1T = persist.tile([128, 2, N], bf16, tag="k1T")
            VW = DHEAD + 1     # 65: per-head v slice width (data + ones col)
            v_sb = persist.tile([128, NT, H_LOC * VW], bf16, tag="v")
            v1_sb = persist.tile([128, NT, H_LOC * VW], bf16, tag="v1")
            for vt in (v_sb, v1_sb):
                nc.vector.tensor_copy(
                    vt[:].rearrange("p t (h c) -> p t h c", h=H_LOC)
                    [:, :, :, DHEAD:],
                    ones_f32[:, None, None, :].broadcast_to(
                        [128, NT, H_LOC, 1]),
                )

            # x staging: 4 persistent tiles, DMA'd up front, spread across
            # engines' DMA queues; x1 h0 first (earliest consumer).
            xstages = {}
            for srci in range(2):
                for half in range(2):
                    xstages[(srci, half)] = persist.tile(
                        [128, KO, 1024], bf16, tag=f"xs{srci}{half}",
                        name=f"xs{srci}{half}")
            x1r = x1T.rearrange("(ko p) n -> p ko n", p=128)
            xr = xT.rearrange("(ko p) n -> p ko n", p=128)
            nc.sync.dma_start(xstages[(1, 0)][:], x1r[:, :, 0:1024])
            nc.scalar.dma_start(xstages[(1, 1)][:], x1r[:, :, 1024:2048])
            nc.scalar.dma_start(wqkv_sb[:, :, 512:768], wqkv_r[:, :, 512:768])
            nc.scalar.dma_start(
                wout_sb[:], wout.rearrange("(ki p) d -> p ki d", p=128))
            nc.sync.dma_start(xstages[(0, 0)][:], xr[:, :, 0:1024])
            nc.sync.dma_start(xstages[(0, 1)][:], xr[:, :, 1024:2048])

            QDST = {0: qT, 1: q1T}
            KDST = {0: kT, 1: k1T}

            emitted = set()

            def qkv_qk(srci, half, mb, nbh):
                """q (mb 0,1) / k (mb 2,3) chunk for 512 tokens."""
                key = ("qk", srci, half, mb, nbh)
                if key in emitted:
                    return
                emitted.add(key)
                xs = xstages[(srci, half)]
                dst = QDST[srci] if mb < 2 else KDST[srci]
                ci = mb % 2
                ps_qk = ps_mix.tile([128, 512], f32, tag="mix", name="ps_qk")
                for ko in range(KO):
                    nc.tensor.matmul(
                        ps_qk[:],
                        wqkv_sb[:, ko, mb * 128:(mb + 1) * 128],
                        xs[:, ko, nbh * 512:(nbh + 1) * 512],
                        start=(ko == 0),
                        stop=(ko == KO - 1),
                    )
                nc.vector.tensor_copy(
                    dst[:, ci, half * 1024 + nbh * 512:
                        half * 1024 + (nbh + 1) * 512],
                    ps_qk[:],
                )

            def qkv_v(srci, half, nt):
                """v tile for one 128-token m chunk (all 4 heads)."""
                key = ("v", srci, half, nt)
                if key in emitted:
                    return
                emitted.add(key)
                xs = xstages[(srci, half)]
                vdst = v_sb if srci == 0 else v1_sb
                ps_v = ps_mix.tile([128, 256], f32, tag="mix", name="ps_v")
                for ko in range(KO):
                    nc.tensor.matmul(
                        ps_v[:],
                        xs[:, ko, nt * 128:(nt + 1) * 128],
                        wqkv_sb[:, ko, 2 * INNER_LOC:3 * INNER_LOC],
                        start=(ko == 0),
                        stop=(ko == KO - 1),
                    )
                nc.vector.tensor_copy(
                    vdst[:, half * 8 + nt, :]
                    .rearrange("p (h c) -> p h c", h=H_LOC)[:, :, :DHEAD],
                    ps_v[:].rearrange("p (h c) -> p h c", h=H_LOC),
                )

            ots = {}

            def proj_nt(nb, d, nt):
                """out-projection for one 128-row chunk."""
                ot = ots[(nb, d)]
                ob = obp.tile([128, DIM], f32, tag="ob", name="ob")
                for db in range(2):
                    ps_o = ps_mix.tile([128, 512], f32, tag="mix", name="ps_o")
                    for ki in range(2):
                        nc.tensor.matmul(
                            ps_o[:],
                            ot[:, ki, nt * 128:(nt + 1) * 128],
                            wout_sb[:, ki, db * 512:(db + 1) * 512],
                            start=(ki == 0),
                            stop=(ki == 1),
                        )
                    nc.vector.tensor_copy(ob[:, db * 512:(db + 1) * 512],
                                          ps_o[:])
                nc.sync.dma_start(
                    out[d, nb * 512 + nt * 128:nb * 512 + (nt + 1) * 128, :],
                    ob[:],
                )

            # ---------------- filler machinery ----------------
            filler = collections.OrderedDict()
            filler_left = [0]

            def F(label, fns, cost):
                filler[label] = (fns, cost)
                filler_left[0] += cost

            budget = 0

            def emit_filler(label):
                if label not in filler:
                    return 0
                fns, cost = filler.pop(label)
                filler_left[0] -= cost
                for fn in fns:
                    fn()
                return cost

            def pump(extra):
                nonlocal budget
                budget += extra
                while filler:
                    label, (fns, cost) = next(iter(filler.items()))
                    if cost > budget:
                        break
                    budget -= cost
                    filler.pop(label)
                    filler_left[0] -= cost
                    for fn in fns:
                        fn()

            def need(labels):
                nonlocal budget
                for lb in labels:
                    budget -= emit_filler(lb)
                budget = max(budget, -40000)

            def L_qk(srci, half, mb):
                return [lambda nbh=nbh: qkv_qk(srci, half, mb, nbh)
                        for nbh in range(2)]

            def L_v(srci, half, lo=0, hi=8):
                return [lambda nt=nt: qkv_v(srci, half, nt)
                        for nt in range(lo, hi)]

            # filler order: things needed soonest first
            F("x1h0.k0b", [lambda: qkv_qk(1, 0, 2, 1)], 4096)
            F("x1h0.va", L_v(1, 0, 0, 4), 4096)
            F("x1h0.vb", L_v(1, 0, 4, 8), 4096)
            F("x1h1.k0", L_qk(1, 1, 2), 8192)
            F("x1h1.v", L_v(1, 1), 8192)
            F("x1h0.k1", L_qk(1, 0, 3), 8192)
            F("x1h1.k1", L_qk(1, 1, 3), 8192)
            F("xh0.q", L_qk(0, 0, 0) + L_qk(0, 0, 1), 16384)
            F("xh1.q", L_qk(0, 1, 0) + L_qk(0, 1, 1), 16384)
            F("xh0.k", L_qk(0, 0, 2) + L_qk(0, 0, 3), 16384)
            F("xh0.v", L_v(0, 0), 8192)
            F("xh1.k", L_qk(0, 1, 2) + L_qk(0, 1, 3), 16384)
            F("xh1.v", L_v(0, 1), 8192)
            F("x1h0.q", L_qk(1, 0, 0) + L_qk(1, 0, 1), 16384)
            F("x1h1.q", L_qk(1, 1, 0) + L_qk(1, 1, 1), 16384)

            def unit_needs(nb, d, pair):
                if d == 0:
                    kv0 = ["x1h0.k1"] if pair else []  # pair0 h0 is prefix
                    kv1 = ["x1h1.k1"] if pair else ["x1h1.k0", "x1h1.v"]
                    q = ["xh0.q" if nb < 2 else "xh1.q"]
                else:
                    kv0 = [] if pair else ["xh0.k", "xh0.v"]
                    kv1 = [] if pair else ["xh1.k", "xh1.v"]
                    q = ["x1h0.q" if nb < 2 else "x1h1.q"]
                return kv0, kv1, q

            # ---------------- attention ----------------
            state = {}
            combo_hist = []

            def attn_main(nb, d, pair):
                nslc = slice(nb * 512, (nb + 1) * 512)
                qsrc, ksrc, vsrc = (
                    (qT, k1T, v1_sb) if d == 0 else (q1T, kT, v_sb)
                )
                kv0, kv1, qn = unit_needs(nb, d, pair)
                need(kv0 + qn)
                if (nb, d) not in ots:
                    # otp has 3 bufs; before taking a 4th-back slot, force the
                    # 3-back combo's proj fillers so the WAR order is valid.
                    combo_hist.append((nb, d))
                    if len(combo_hist) > 3:
                        pnb, pd = combo_hist[-4]
                        need([f"proj{pnb}{pd}{nt}" for nt in range(4)])
                    ots[(nb, d)] = otp.tile([128, 2, 512], bf16, tag="ot",
                                            name="ot")
                poA = ps_av.tile([VW, 512], f32, tag="poA", bufs=1, name="poA")
                poB = ps_av.tile([VW, 512], f32, tag="poB", bufs=1, name="poB")
                atiles = [None] * NT

                def scores(mc):
                    ps = ps_s.tile([128, 1024], f32, tag="ps_s", name="ps_s")
                    m0 = mc * 128
                    if QUAD_SCORES:
                        nc.tensor.matmul(
                            ps[0:64, 0:512],
                            ksrc[0:64, pair, m0:m0 + 64],
                            qsrc[0:64, pair, nslc],
                            start=True, stop=True, tile_position=(0, 0))
                        nc.tensor.matmul(
                            ps[64:128, 0:512],
                            ksrc[0:64, pair, m0 + 64:m0 + 128],
                            qsrc[0:64, pair, nslc],
                            start=True, stop=True, tile_position=(0, 64))
                        nc.tensor.matmul(
                            ps[0:64, 512:1024],
                            ksrc[64:128, pair, m0:m0 + 64],
                            qsrc[64:128, pair, nslc],
                            start=True, stop=True, tile_position=(64, 0))
                        nc.tensor.matmul(
                            ps[64:128, 512:1024],
                            ksrc[64:128, pair, m0 + 64:m0 + 128],
                            qsrc[64:128, pair, nslc],
                            start=True, stop=True, tile_position=(64, 64))
                    else:
                        nc.tensor.matmul(
                            ps[:, 0:512],
                            ksrc[0:64, pair, m0:m0 + 128],
                            qsrc[0:64, pair, nslc],
                            start=True, stop=True)
                        nc.tensor.matmul(
                            ps[:, 512:1024],
                            ksrc[64:128, pair, m0:m0 + 128],
                            qsrc[64:128, pair, nslc],
                            start=True, stop=True)
                    a = apool.tile([128, 1024], bf16, tag="a", name="a")
                    nc.scalar.activation(a[:], ps[:], AF.Exp, scale=SCALE)
                    atiles[mc] = a

                def av(mc):
                    a = atiles[mc]
                    kw = dict(start=(mc == 0), stop=(mc == NT - 1))
                    nc.tensor.matmul(
                        poA[:],
                        vsrc[:, mc, (2 * pair) * VW:(2 * pair + 1) * VW],
                        a[:, 0:512], **kw)
                    nc.tensor.matmul(
                        poB[:],
                        vsrc[:, mc, (2 * pair + 1) * VW:(2 * pair + 2) * VW],
                        a[:, 512:1024], **kw)
                    atiles[mc] = None

                # scores emitted in chunk pairs: batching amortizes the
                # (64,64)<->(128,128) array-mode switch, but groups larger
                # than the 2 psum score buffers would stall PE in-order with
                # nothing runnable between the quads.
                GRP, LAG = 2, 4
                first = d == 0 and pair == 0 and nb == 0
                for g in range(NT // GRP + LAG // GRP):
                    mc0 = GRP * g
                    if mc0 < NT:
                        if mc0 == 2 and first:
                            need(["x1h0.k0b", "x1h0.va"])
                        if mc0 == 6 and first:
                            need(["x1h0.vb"])
                        if mc0 == 8:
                            need(kv1)
                        for i in range(GRP):
                            scores(mc0 + i)
                    for i in range(GRP):
                        mc = GRP * g - LAG + i
                        if 0 <= mc < NT:
                            av(mc)
                    yield
                state[(nb, d, pair)] = (poA, poB)

            def attn_tail(nb, d, pair):
                poA, poB = state.pop((nb, d, pair))
                ot = ots[(nb, d)]
                for i, poX in enumerate((poA, poB)):
                    prow = slice(i * 64, i * 64 + 64)
                    cs = miscp.tile([1, 512], f32, tag="cs", name="cs")
                    nc.vector.tensor_copy(cs[:], poX[64:65, :])
                    rcf = miscp.tile([1, 512], f32, tag="rcf", name="rcf")
                    nc.vector.reciprocal_approx_fast(out=rcf[:], in_=cs[:])
                    rcb = miscp.tile([1, 512], bf16, tag="rcb", name="rcb")
                    nc.vector.tensor_copy(rcb[:], rcf[:])
                    pbX = ps_mix.tile([64, 512], f32, tag="mix", name="pb")
                    nc.tensor.matmul(pbX[:], ones_row[:], rcb[:],
                                     start=True, stop=True)
                    nc.vector.tensor_copy(ot[prow, pair, :], poX[0:64, :])
                    nc.vector.tensor_mul(ot[prow, pair, :],
                                         ot[prow, pair, :], pbX[:])
                    yield
                if pair == 1:
                    for nt in range(4):
                        F(f"proj{nb}{d}{nt}",
                          [lambda nt=nt: proj_nt(nb, d, nt)], 2048)
                yield

            # ---------------- warmup + forced prefix ----------------
            # ~3.5us of junk matmuls (results unused) while the x DMAs are
            # in flight: lifts the PE HAM throttle so the prefix QKV and the
            # first attention unit run at 2.4 GHz instead of 1.2.
            for w in range(2):
                junk = ps_s.tile([128, 1024], f32, tag="ps_s", name="junk")
                for ko in range(KO):
                    nc.tensor.matmul(junk[:, 0:512],
                                     wqkv_sb[:, ko, 0:128],
                                     wqkv_sb[:, 0, 0:512],
                                     start=True, stop=True)
            qkv_qk(1, 0, 2, 0)         # k1 heads 0/1, m 0:512
            qkv_qk(0, 0, 0, 0)         # q heads 0/1, n 0:512

            # ---------------- main schedule ----------------
            UNITS = ([(nb, 0, p) for nb in range(4) for p in range(2)] +
                     [(nb, 1, p) for nb in range(4) for p in range(2)])
            # QKV fillers must finish during the dir-0 half (dir-1 units
            # force them otherwise); the back half only drains proj fillers.
            cur_rate = [2700]

            def paced_pump(boost=0):
                pump(cur_rate[0] + boost)

            prev_tail = None
            for ui, unit in enumerate(UNITS):
                cur_rate[0] = 2000 if ui < 8 else 900
                g = attn_main(*unit)
                for i, _ in enumerate(g):
                    paced_pump(1500 if ui == 0 else 0)
                    if i == 1 and prev_tail is not None:
                        for _ in prev_tail:
                            paced_pump()
                        prev_tail = None
                prev_tail = attn_tail(*unit)
            for _ in prev_tail:
                paced_pump()
            need(list(filler.keys()))

    return nc


def _get_graph():
    if "nc" not in _CACHED:
        nc = _build_graph()
        nc.finalize()
        _CACHED["nc"] = nc
    return _CACHED["nc"]


def _make_in_maps(x, x1, W_qkv, W_out):
    in_maps = []
    for c in range(NCORES):
        b = c // 4
        h0 = (c % 4) * H_LOC
        cols = np.concatenate(
            [W_qkv[:, j * DIM + h0 * DHEAD: j * DIM + (h0 + H_LOC) * DHEAD]
             for j in range(3)],
            axis=1,
        )
        in_maps.append({
            "xT": np.ascontiguousarray(x[b].T).astype(bfloat16),
            "x1T": np.ascontiguousarray(x1[b].T).astype(bfloat16),
            "wqkv": np.ascontiguousarray(cols).astype(bfloat16),
            "wout": np.ascontiguousarray(
                W_out[h0 * DHEAD:(h0 + H_LOC) * DHEAD, :]
            ).astype(bfloat16),
        })
    return in_maps


def _run(x, x1, W_qkv, W_out, b_out, **spmd_kwargs):
    from concourse.bass_utils import run_bass_kernel_spmd

    nc = _get_graph()
    in_maps = _make_in_maps(x, x1, W_qkv, W_out)
    res = run_bass_kernel_spmd(nc, in_maps, core_ids=list(range(NCORES)),
                               **spmd_kwargs)
    parts = [r["out"].reshape(2, N, DIM) for r in res.results]
    out = np.zeros((B, N, DIM), np.float32)
    out1 = np.zeros((B, N, DIM), np.float32)
    for b in range(B):
        grp = parts[4 * b:4 * b + 4]
        out[b] = sum(p[0] for p in grp) + b_out
        out1[b] = sum(p[1] for p in grp) + b_out
    return (out, out1), res


def kernel(x, x1, W_qkv, W_out, b_out):
    x = np.asarray(x, np.float32)
    x1 = np.asarray(x1, np.float32)
    W_qkv = np.asarray(W_qkv, np.float32)
    W_out = np.asarray(W_out, np.float32)
    b_out = np.asarray(b_out, np.float32)
    (out, out1), _ = _run(x, x1, W_qkv, W_out, b_out)
    return out, out1
